# revision 7
# baseline (speedup 1.0000x reference)
"""Trainium2 Bass kernel for nn_NodeEdge (gnn_message_passing).

Computes out[b] = (w * inci + b) @ x[b] : [N,E] x [B,E,F] -> [B,N,F]
with N=4096, E=8192, F=256, B=16 (all fp32).

Strategy (8 NeuronCores):
  - Shard the CONTRACTION dim E across the 8 cores (1024 edges each).
    Each core writes a bf16 partial output [B, F, N]; the host sums the
    8 partials in fp32 and transposes to [B, N, F].
  - All heavy inputs are pre-packed on the host into PARTITION-MAJOR
    layouts (one contiguous run per SBUF partition per DMA) so each
    dma_start emits 128 descriptors.  HWDGE descriptor generation is
    serialized on the Sync engine (~3ns/desc + ~0.6us fixed), so w/b/inci
    are additionally packed into a SINGLE interleaved W|B|I byte tensor
    per node block: one DMA emit instead of three.
  - mT = wT*iT + bT is built by VectorE directly in matmul-rhs layout
    [e, n] from the host-transposed staging (no PE transposes).  x ships
    bf16 (halves the startup-critical bytes).
  - Matmuls are bf16 x bf16 -> fp32 PSUM, [128x128]@[128x512] streaming
    at the pure rate (~216ns each, LDWEIGHTS hidden in the background
    weight buffer).  Per node-block of 512: 256 matmuls (16 batches x
    2 f-tiles x 8 e-tiles accumulated in PSUM), ScalarE drains with a
    f32->bf16 cast, DMA out.
  - Startup: node-block 0 is staged in n-QUARTERS and its first batches
    run as half-width (256-col) accumulation groups, so the PE starts as
    soon as ~2MB of supply has landed instead of waiting for the full
    first block.  Each half-width group still owns a full PSUM bank
    (matmul start=True clears the whole bank).
"""

import numpy as np
import ml_dtypes

N, E, F, B = 4096, 8192, 256, 16
NCORES = 8
ESH = E // NCORES      # 1024 contraction elements per core
ET = ESH // 128        # 8 e-tiles per core
NBLK = 512             # node-block (output columns per psum accumulation)
FT = F // 256 * 2      # 2 f-tiles of 128
NJ = N // NBLK         # 8 node blocks

_CACHE = {}


def _build_nc():
    import concourse.mybir as mybir
    import concourse.tile as tile
    from concourse import bacc

    f32 = mybir.dt.float32
    bf16 = mybir.dt.bfloat16
    u8 = mybir.dt.uint8

    nc = bacc.Bacc(None, target_bir_lowering=False)
    # Host-packed layouts (partition dim first, one contiguous run per
    # partition per DMA):
    #   x[p, b, et*F+f] = x[b, et*128+p, f]                       (bf16)
    #   wbi[p, j-1, :]  = w.T | b.T | inci.T bytes for node block j
    #                     each in (et, n512) order               (j = 1..7)
    #   wbi0[p, q, :]   = same for node block 0, n-quarter q (et, n128)
    x_d = nc.dram_tensor("x", [128, B, ET * F], bf16, kind="ExternalInput")
    wbi_d = nc.dram_tensor("wbi", [128, NJ - 1, 5 * ET * NBLK], u8, kind="ExternalInput")
    wbi0_d = nc.dram_tensor("wbi0", [128, 4, 5 * ET * 128], u8, kind="ExternalInput")
    o_d = nc.dram_tensor("out", [B, F, N], bf16, kind="ExternalOutput")

    with tile.TileContext(nc) as tc:
        with (
            tc.tile_pool(name="xres", bufs=1) as xpool,
            tc.tile_pool(name="mtp", bufs=2) as mtpool,
            tc.tile_pool(name="stg", bufs=3) as stgpool,
            tc.tile_pool(name="op", bufs=8) as opool,
            tc.tile_pool(name="mm", bufs=7, space="PSUM") as mmpool,
        ):
            XG = 2  # batches per x DMA group
            xgs = [None] * (B // XG)
            mts = [None] * NJ

            def load_x(q):
                xt = xpool.tile([128, XG * ET * F], bf16, tag=f"x{q}", name=f"x_sb{q}")
                nc.sync.dma_start(
                    out=xt[:],
                    in_=x_d[:, q * XG : (q + 1) * XG].rearrange("p b c -> p (b c)"),
                )
                xgs[q] = xt

            def x_slice(bb, c0):
                return xgs[bb // XG][:, (bb % XG) * ET * F + c0 : (bb % XG) * ET * F + c0 + 128]

            def alloc_mt(j):
                mts[j] = mtpool.tile([128, ET * NBLK], bf16, tag="mt", name=f"mt{j}")

            def prep_full(j):
                # One packed DMA + two full-width contiguous VectorE ops.
                alloc_mt(j)
                st = stgpool.tile([128, 5 * 4096], u8, tag="stg", name=f"st{j}")
                nc.sync.dma_start(out=st[:], in_=wbi_d[:, j - 1])
                wv = st[:, 0:8192].bitcast(bf16)
                bv = st[:, 8192:16384].bitcast(bf16)
                iv = st[:, 16384:20480]
                nc.vector.tensor_mul(out=mts[j][:], in0=wv, in1=iv)
                nc.vector.tensor_add(out=mts[j][:], in0=mts[j][:], in1=bv)

            def prep0_q(q):
                # Node block 0, n-quarter q: small packed DMA so mT[0]
                # becomes usable piecewise as bytes land.
                st = stgpool.tile([128, 5 * 1024], u8, tag="stg0", name=f"st0_{q}")
                nc.sync.dma_start(out=st[:], in_=wbi0_d[:, q])
                wv = st[:, 0:2048].bitcast(bf16).rearrange("p (et n) -> p et n", n=128)
                bv = st[:, 2048:4096].bitcast(bf16).rearrange("p (et n) -> p et n", n=128)
                iv = st[:, 4096:5120].rearrange("p (et n) -> p et n", n=128)
                dst = mts[0].rearrange("p (et n) -> p et n", n=NBLK)[
                    :, :, q * 128 : (q + 1) * 128
                ]
                nc.vector.tensor_mul(out=dst, in0=wv, in1=iv)
                nc.vector.tensor_add(out=dst, in0=dst, in1=bv)

            def group(j, bb, ft, n0, n1, tail_split=False):
                ps = mmpool.tile([128, NBLK], f32, tag="ps", name=f"ps{j}_{bb}_{ft}_{n0}")
                for et in range(ET):
                    c0 = et * F + ft * 128
                    nc.tensor.matmul(
                        ps[:, n0:n1],
                        lhsT=x_slice(bb, c0),
                        rhs=mts[j][:, et * NBLK + n0 : et * NBLK + n1],
                        start=(et == 0),
                        stop=(et == ET - 1),
                    )
                ot = opool.tile([128, NBLK], bf16, tag="o", name=f"o{j}_{bb}_{ft}_{n0}")
                orow = o_d[bb, ft * 128 : (ft + 1) * 128]
                if tail_split:
                    # Pipeline the final drain: half-drain, half-DMA-out, so
                    # the kernel tail is half a drain shorter.
                    mid = (n0 + n1) // 2
                    nc.scalar.copy(out=ot[:, n0:mid], in_=ps[:, n0:mid])
                    nc.sync.dma_start(
                        out=orow[:, j * NBLK + n0 : j * NBLK + mid], in_=ot[:, n0:mid]
                    )
                    nc.scalar.copy(out=ot[:, mid:n1], in_=ps[:, mid:n1])
                    nc.sync.dma_start(
                        out=orow[:, j * NBLK + mid : j * NBLK + n1], in_=ot[:, mid:n1]
                    )
                else:
                    nc.scalar.copy(out=ot[:, n0:n1], in_=ps[:, n0:n1])
                    nc.sync.dma_start(
                        out=orow[:, j * NBLK + n0 : j * NBLK + n1], in_=ot[:, n0:n1]
                    )

            def mms(j, b_lo=0, b_hi=B):
                for bb in range(b_lo, b_hi):
                    for ft in range(FT):
                        last = j == NJ - 1 and bb == B - 1 and ft == FT - 1
                        group(j, bb, ft, 0, NBLK, tail_split=last)

            # ---- software pipeline ----
            load_x(0)
            alloc_mt(0)
            prep0_q(0)
            prep0_q(1)
            load_x(1)
            prep0_q(2)
            prep0_q(3)
            load_x(2)
            prep_full(1)
            for q in range(3, B // XG):
                load_x(q)
            # Node block 0: first batches run half-width so matmuls start as
            # soon as x group 0 + quarters 0-1 have landed.
            for bb in range(4):
                for ft in range(FT):
                    group(0, bb, ft, 0, 256)
                    group(0, bb, ft, 256, NBLK)
            for bb in range(4, B):
                for ft in range(FT):
                    group(0, bb, ft, 0, NBLK)
            prep_full(2)
            mms(1)
            prep_full(3)
            for j in range(2, NJ):
                mms(j)
                if j + 2 < NJ:
                    prep_full(j + 2)
    nc.finalize()
    return nc


def _get_nc():
    if "nc" not in _CACHE:
        _CACHE["nc"] = _build_nc()
    return _CACHE["nc"]


def run(inputs, trace=False, tmpdir=None, trace_cores=None):
    """Shard + host-pack inputs, run the SPMD bass kernel on 8 cores,
    return (full_output, BassKernelResults)."""
    from concourse.bass_utils import run_bass_kernel_spmd

    bf16 = ml_dtypes.bfloat16
    x = np.asarray(inputs["x"], dtype=np.float32)
    w = np.asarray(inputs["w"], dtype=np.float32)
    inci = np.asarray(inputs["inci"], dtype=np.float32)
    b = np.asarray(inputs["b"], dtype=np.float32)
    assert x.shape == (B, E, F) and w.shape == (N, E)

    in_maps = []
    for c in range(NCORES):
        sl = slice(c * ESH, (c + 1) * ESH)
        wT = w[:, sl].T  # [ESH, N]
        bT = b[:, sl].T
        iT = inci[:, sl].T

        def block(a, j0, j1, nsub, dt):
            # [ESH, ncols] -> [128, nblocks, et-major bytes]
            t = a[:, j0:j1]
            nb = (j1 - j0) // nsub
            t = t.reshape(ET, 128, nb, nsub).transpose(1, 2, 0, 3)  # [128, nb, et, nsub]
            t = np.ascontiguousarray(t.astype(dt))
            return t.reshape(128, nb, -1).view(np.uint8)

        # node blocks 1..7: w|b|i packed per block, (et, n512) order
        wbi = np.concatenate(
            [
                block(wT, NBLK, N, NBLK, bf16),
                block(bT, NBLK, N, NBLK, bf16),
                block(iT, NBLK, N, NBLK, np.uint8),
            ],
            axis=2,
        )
        # node block 0 in quarters, (et, n128) order
        wbi0 = np.concatenate(
            [
                block(wT, 0, NBLK, 128, bf16),
                block(bT, 0, NBLK, 128, bf16),
                block(iT, 0, NBLK, 128, np.uint8),
            ],
            axis=2,
        )
        # x[p, b, et*F+f] = x[b, et*128+p, f]
        xp = np.ascontiguousarray(
            x[:, sl, :].reshape(B, ET, 128, F).transpose(2, 0, 1, 3).astype(bf16)
        ).reshape(128, B, ET * F)
        in_maps.append({"x": xp, "wbi": wbi, "wbi0": wbi0})

    nc = _get_nc()
    res = run_bass_kernel_spmd(
        nc,
        in_maps,
        core_ids=list(range(NCORES)),
        trace=trace,
        tmpdir=tmpdir,
        trace_cores=trace_cores,
    )
    # Sum the 8 bf16 partial products in fp32 and transpose [B,F,N]->[B,N,F].
    total = res.results[0]["out"].astype(np.float32)
    for c in range(1, NCORES):
        total = total + res.results[c]["out"].astype(np.float32)
    out = np.ascontiguousarray(total.transpose(0, 2, 1))
    return out, res


def kernel(x, inci, w, b):
    out, _ = run({"x": x, "inci": inci, "w": w, "b": b})
    return out


# revision 10
# speedup vs baseline: 1.0382x; 1.0382x over previous
"""Trainium2 Bass kernel for nn_NodeEdge (gnn_message_passing).

Computes out[b] = (w * inci + b) @ x[b] : [N,E] x [B,E,F] -> [B,N,F]
with N=4096, E=8192, F=256, B=16 (all fp32).

Strategy (8 NeuronCores):
  - Shard the CONTRACTION dim E across the 8 cores (1024 edges each).
    Each core writes a bf16 partial output [B, F, N]; the host sums the
    8 partials in fp32 and transposes to [B, N, F].
  - All heavy inputs are pre-packed on the host into PARTITION-MAJOR
    layouts (one contiguous run per SBUF partition per DMA) so each
    dma_start emits 128 descriptors.  HWDGE descriptor generation is
    serialized on the Sync engine (~3ns/desc + ~0.6us fixed), so w/b/inci
    are additionally packed into a SINGLE interleaved W|B|I byte tensor
    per node block: one DMA emit instead of three.
  - mT = wT*iT + bT is built by VectorE directly in matmul-rhs layout
    [e, n] from the host-transposed staging (no PE transposes).  x ships
    bf16 (halves the startup-critical bytes).
  - Matmuls are bf16 x bf16 -> fp32 PSUM, [128x128]@[128x512] streaming
    at the pure rate (~216ns each, LDWEIGHTS hidden in the background
    weight buffer).  Per node-block of 512: 256 matmuls (16 batches x
    2 f-tiles x 8 e-tiles accumulated in PSUM), ScalarE drains with a
    f32->bf16 cast, DMA out.
  - Startup: node-block 0 is staged in n-QUARTERS and its first batches
    run as half-width (256-col) accumulation groups, so the PE starts as
    soon as ~2MB of supply has landed instead of waiting for the full
    first block.  Each half-width group still owns a full PSUM bank
    (matmul start=True clears the whole bank).
"""

import numpy as np
import ml_dtypes

N, E, F, B = 4096, 8192, 256, 16
NCORES = 8
ESH = E // NCORES      # 1024 contraction elements per core
ET = ESH // 128        # 8 e-tiles per core
NBLK = 512             # node-block (output columns per psum accumulation)
FT = F // 256 * 2      # 2 f-tiles of 128
NJ = N // NBLK         # 8 node blocks

_CACHE = {}


def _build_nc():
    import concourse.mybir as mybir
    import concourse.tile as tile
    from concourse import bacc

    f32 = mybir.dt.float32
    bf16 = mybir.dt.bfloat16
    u8 = mybir.dt.uint8

    nc = bacc.Bacc(None, target_bir_lowering=False)
    # Host-packed layouts (partition dim first, one contiguous run per
    # partition per DMA):
    #   x[p, b, et*F+f] = x[b, et*128+p, f]                       (bf16)
    #   wbi[p, j-1, :]  = w.T | b.T | inci.T bytes for node block j
    #                     each in (et, n512) order               (j = 1..7)
    #   wbi0[p, q, :]   = same for node block 0, n-quarter q (et, n128)
    x_d = nc.dram_tensor("x", [128, B, ET * F], bf16, kind="ExternalInput")
    wbi_d = nc.dram_tensor("wbi", [128, NJ - 1, 5 * ET * NBLK], u8, kind="ExternalInput")
    wbi0_d = nc.dram_tensor("wbi0", [128, 4, 5 * ET * 128], u8, kind="ExternalInput")
    o_d = nc.dram_tensor("out", [B, F, N], bf16, kind="ExternalOutput")

    with tile.TileContext(nc) as tc:
        with (
            tc.tile_pool(name="xres", bufs=1) as xpool,
            tc.tile_pool(name="mtp", bufs=2) as mtpool,
            tc.tile_pool(name="stg", bufs=3) as stgpool,
            tc.tile_pool(name="op", bufs=24) as opool,
            tc.tile_pool(name="mm", bufs=7, space="PSUM") as mmpool,
            tc.tile_pool(name="wp", bufs=1, space="PSUM") as warmpool,
        ):
            XG = 2  # batches per x DMA group
            xgs = [None] * (B // XG)
            mts = [None] * NJ

            def load_x(q):
                xt = xpool.tile([128, XG * ET * F], bf16, tag=f"x{q}", name=f"x_sb{q}")
                nc.sync.dma_start(
                    out=xt[:],
                    in_=x_d[:, q * XG : (q + 1) * XG].rearrange("p b c -> p (b c)"),
                )
                xgs[q] = xt

            def x_slice(bb, c0):
                return xgs[bb // XG][:, (bb % XG) * ET * F + c0 : (bb % XG) * ET * F + c0 + 128]

            def alloc_mt(j):
                mts[j] = mtpool.tile([128, ET * NBLK], bf16, tag="mt", name=f"mt{j}")

            def prep_full(j):
                # One packed DMA + two full-width contiguous VectorE ops.
                alloc_mt(j)
                st = stgpool.tile([128, 5 * 4096], u8, tag="stg", name=f"st{j}")
                nc.sync.dma_start(out=st[:], in_=wbi_d[:, j - 1])
                wv = st[:, 0:8192].bitcast(bf16)
                bv = st[:, 8192:16384].bitcast(bf16)
                iv = st[:, 16384:20480]
                nc.vector.tensor_mul(out=mts[j][:], in0=wv, in1=iv)
                nc.vector.tensor_add(out=mts[j][:], in0=mts[j][:], in1=bv)

            def prep0_q(q):
                # Node block 0, n-quarter q: small packed DMA so mT[0]
                # becomes usable piecewise as bytes land.
                st = stgpool.tile([128, 5 * 1024], u8, tag="stg0", name=f"st0_{q}")
                nc.sync.dma_start(out=st[:], in_=wbi0_d[:, q])
                wv = st[:, 0:2048].bitcast(bf16).rearrange("p (et n) -> p et n", n=128)
                bv = st[:, 2048:4096].bitcast(bf16).rearrange("p (et n) -> p et n", n=128)
                iv = st[:, 4096:5120].rearrange("p (et n) -> p et n", n=128)
                dst = mts[0].rearrange("p (et n) -> p et n", n=NBLK)[
                    :, :, q * 128 : (q + 1) * 128
                ]
                nc.vector.tensor_mul(out=dst, in0=wv, in1=iv)
                nc.vector.tensor_add(out=dst, in0=dst, in1=bv)

            def group(j, bb, ft, n0, n1, tail_split=False):
                ps = mmpool.tile([128, NBLK], f32, tag="ps", name=f"ps{j}_{bb}_{ft}_{n0}")
                for et in range(ET):
                    c0 = et * F + ft * 128
                    nc.tensor.matmul(
                        ps[:, n0:n1],
                        lhsT=x_slice(bb, c0),
                        rhs=mts[j][:, et * NBLK + n0 : et * NBLK + n1],
                        start=(et == 0),
                        stop=(et == ET - 1),
                    )
                ot = opool.tile([128, NBLK], bf16, tag="o", name=f"o{j}_{bb}_{ft}_{n0}")
                orow = o_d[bb, ft * 128 : (ft + 1) * 128]
                if tail_split:
                    # Pipeline the final drain: half-drain, half-DMA-out, so
                    # the kernel tail is half a drain shorter.
                    mid = (n0 + n1) // 2
                    nc.scalar.copy(out=ot[:, n0:mid], in_=ps[:, n0:mid])
                    nc.sync.dma_start(
                        out=orow[:, j * NBLK + n0 : j * NBLK + mid], in_=ot[:, n0:mid]
                    )
                    nc.scalar.copy(out=ot[:, mid:n1], in_=ps[:, mid:n1])
                    nc.sync.dma_start(
                        out=orow[:, j * NBLK + mid : j * NBLK + n1], in_=ot[:, mid:n1]
                    )
                else:
                    nc.scalar.copy(out=ot[:, n0:n1], in_=ps[:, n0:n1])
                    nc.sync.dma_start(
                        out=orow[:, j * NBLK + n0 : j * NBLK + n1], in_=ot[:, n0:n1]
                    )

            def mms(j, b_lo=0, b_hi=B):
                for bb in range(b_lo, b_hi):
                    for ft in range(FT):
                        last = j == NJ - 1 and bb == B - 1 and ft == FT - 1
                        group(j, bb, ft, 0, NBLK, tail_split=last)

            # ---- software pipeline ----
            # PE pre-warm first (no input deps): dummy matmuls on a zeroed
            # tile keep the PE busy through the HAM activity window while the
            # first real supply streams in, so the real matmuls start at
            # K=8/8 (2.4 GHz) instead of paying the 1.2 GHz cold ramp.
            wz = stgpool.tile([128, 128], bf16, tag="warm", name="wz")
            nc.vector.memset(wz[:], 0.0)
            ps_warm = warmpool.tile([128, 128], f32, tag="psw", name="ps_warm")
            for _ in range(55):
                nc.tensor.matmul(ps_warm[:], lhsT=wz[:], rhs=wz[:], start=True, stop=True)
            load_x(0)
            alloc_mt(0)
            prep0_q(0)
            prep0_q(1)
            load_x(1)
            prep0_q(2)
            prep0_q(3)
            load_x(2)
            prep_full(1)
            for q in range(3, B // XG):
                load_x(q)
            # Node block 0: first batches run half-width so matmuls start as
            # soon as x group 0 + quarters 0-1 have landed; the n-halves are
            # swept separately so the second half only needs quarters 2-3.
            for bb in range(4):
                for ft in range(FT):
                    group(0, bb, ft, 0, 256)
            for bb in range(4):
                for ft in range(FT):
                    group(0, bb, ft, 256, NBLK)
            for bb in range(4, B):
                for ft in range(FT):
                    group(0, bb, ft, 0, NBLK)
            prep_full(2)
            mms(1)
            prep_full(3)
            for j in range(2, NJ):
                mms(j)
                if j + 2 < NJ:
                    prep_full(j + 2)
    nc.finalize()
    return nc


def _get_nc():
    if "nc" not in _CACHE:
        _CACHE["nc"] = _build_nc()
    return _CACHE["nc"]


def run(inputs, trace=False, tmpdir=None, trace_cores=None):
    """Shard + host-pack inputs, run the SPMD bass kernel on 8 cores,
    return (full_output, BassKernelResults)."""
    from concourse.bass_utils import run_bass_kernel_spmd

    bf16 = ml_dtypes.bfloat16
    x = np.asarray(inputs["x"], dtype=np.float32)
    w = np.asarray(inputs["w"], dtype=np.float32)
    inci = np.asarray(inputs["inci"], dtype=np.float32)
    b = np.asarray(inputs["b"], dtype=np.float32)
    assert x.shape == (B, E, F) and w.shape == (N, E)

    in_maps = []
    for c in range(NCORES):
        sl = slice(c * ESH, (c + 1) * ESH)
        wT = w[:, sl].T  # [ESH, N]
        bT = b[:, sl].T
        iT = inci[:, sl].T

        def block(a, j0, j1, nsub, dt):
            # [ESH, ncols] -> [128, nblocks, et-major bytes]
            t = a[:, j0:j1]
            nb = (j1 - j0) // nsub
            t = t.reshape(ET, 128, nb, nsub).transpose(1, 2, 0, 3)  # [128, nb, et, nsub]
            t = np.ascontiguousarray(t.astype(dt))
            return t.reshape(128, nb, -1).view(np.uint8)

        # node blocks 1..7: w|b|i packed per block, (et, n512) order
        wbi = np.concatenate(
            [
                block(wT, NBLK, N, NBLK, bf16),
                block(bT, NBLK, N, NBLK, bf16),
                block(iT, NBLK, N, NBLK, np.uint8),
            ],
            axis=2,
        )
        # node block 0 in quarters, (et, n128) order
        wbi0 = np.concatenate(
            [
                block(wT, 0, NBLK, 128, bf16),
                block(bT, 0, NBLK, 128, bf16),
                block(iT, 0, NBLK, 128, np.uint8),
            ],
            axis=2,
        )
        # x[p, b, et*F+f] = x[b, et*128+p, f]
        xp = np.ascontiguousarray(
            x[:, sl, :].reshape(B, ET, 128, F).transpose(2, 0, 1, 3).astype(bf16)
        ).reshape(128, B, ET * F)
        in_maps.append({"x": xp, "wbi": wbi, "wbi0": wbi0})

    nc = _get_nc()
    res = run_bass_kernel_spmd(
        nc,
        in_maps,
        core_ids=list(range(NCORES)),
        trace=trace,
        tmpdir=tmpdir,
        trace_cores=trace_cores,
    )
    # Sum the 8 bf16 partial products in fp32 and transpose [B,F,N]->[B,N,F].
    total = res.results[0]["out"].astype(np.float32)
    for c in range(1, NCORES):
        total = total + res.results[c]["out"].astype(np.float32)
    out = np.ascontiguousarray(total.transpose(0, 2, 1))
    return out, res


def kernel(x, inci, w, b):
    out, _ = run({"x": x, "inci": inci, "w": w, "b": b})
    return out


# revision 11
# speedup vs baseline: 1.0429x; 1.0046x over previous
"""Trainium2 Bass kernel for nn_NodeEdge (gnn_message_passing).

Computes out[b] = (w * inci + b) @ x[b] : [N,E] x [B,E,F] -> [B,N,F]
with N=4096, E=8192, F=256, B=16 (all fp32).

Strategy (8 NeuronCores):
  - Shard the CONTRACTION dim E across the 8 cores (1024 edges each).
    Each core writes a bf16 partial output [B, F, N]; the host sums the
    8 partials in fp32 and transposes to [B, N, F].
  - All heavy inputs are pre-packed on the host into PARTITION-MAJOR
    layouts (one contiguous run per SBUF partition per DMA) so each
    dma_start emits 128 descriptors.  HWDGE descriptor generation is
    serialized on the Sync engine (~3ns/desc + ~0.6us fixed), so w/b/inci
    are additionally packed into a SINGLE interleaved W|B|I byte tensor
    per node block: one DMA emit instead of three.
  - mT = wT*iT + bT is built by VectorE directly in matmul-rhs layout
    [e, n] from the host-transposed staging (no PE transposes).  x ships
    bf16 (halves the startup-critical bytes).
  - Matmuls are bf16 x bf16 -> fp32 PSUM, [128x128]@[128x512] streaming
    at the pure rate (~216ns each, LDWEIGHTS hidden in the background
    weight buffer).  Per node-block of 512: 256 matmuls (16 batches x
    2 f-tiles x 8 e-tiles accumulated in PSUM), ScalarE drains with a
    f32->bf16 cast, DMA out.
  - Startup: node-block 0 is staged in n-QUARTERS and its first batches
    run as half-width (256-col) accumulation groups, so the PE starts as
    soon as ~2MB of supply has landed instead of waiting for the full
    first block.  Each half-width group still owns a full PSUM bank
    (matmul start=True clears the whole bank).
"""

import numpy as np
import ml_dtypes

N, E, F, B = 4096, 8192, 256, 16
NCORES = 8
ESH = E // NCORES      # 1024 contraction elements per core
ET = ESH // 128        # 8 e-tiles per core
NBLK = 512             # node-block (output columns per psum accumulation)
FT = F // 256 * 2      # 2 f-tiles of 128
NJ = N // NBLK         # 8 node blocks

_CACHE = {}


def _build_nc():
    import concourse.mybir as mybir
    import concourse.tile as tile
    from concourse import bacc

    f32 = mybir.dt.float32
    bf16 = mybir.dt.bfloat16
    u8 = mybir.dt.uint8

    nc = bacc.Bacc(None, target_bir_lowering=False)
    # Host-packed layouts (partition dim first, one contiguous run per
    # partition per DMA):
    #   x[p, b, et*F+f] = x[b, et*128+p, f]                       (bf16)
    #   wbi[p, j-1, :]  = w.T | b.T | inci.T bytes for node block j
    #                     each in (et, n512) order               (j = 1..7)
    #   wbi0[p, q, :]   = same for node block 0, n-quarter q (et, n128)
    x_d = nc.dram_tensor("x", [128, B, ET * F], bf16, kind="ExternalInput")
    wbi_d = nc.dram_tensor("wbi", [128, NJ - 1, 5 * ET * NBLK], u8, kind="ExternalInput")
    wbi0_d = nc.dram_tensor("wbi0", [128, 4, 5 * ET * 128], u8, kind="ExternalInput")
    o_d = nc.dram_tensor("out", [B, F, N], bf16, kind="ExternalOutput")

    with tile.TileContext(nc) as tc:
        with (
            tc.tile_pool(name="xres", bufs=1) as xpool,
            tc.tile_pool(name="mtp", bufs=2) as mtpool,
            tc.tile_pool(name="stg", bufs=3) as stgpool,
            tc.tile_pool(name="op", bufs=24) as opool,
            tc.tile_pool(name="mm", bufs=7, space="PSUM") as mmpool,
            tc.tile_pool(name="wp", bufs=1, space="PSUM") as warmpool,
        ):
            XG = 2  # batches per x DMA group
            xgs = [None] * (B // XG)
            mts = [None] * NJ

            def load_x(q):
                xt = xpool.tile([128, XG * ET * F], bf16, tag=f"x{q}", name=f"x_sb{q}")
                nc.sync.dma_start(
                    out=xt[:],
                    in_=x_d[:, q * XG : (q + 1) * XG].rearrange("p b c -> p (b c)"),
                )
                xgs[q] = xt

            def x_slice(bb, c0):
                return xgs[bb // XG][:, (bb % XG) * ET * F + c0 : (bb % XG) * ET * F + c0 + 128]

            def alloc_mt(j):
                mts[j] = mtpool.tile([128, ET * NBLK], bf16, tag="mt", name=f"mt{j}")

            def prep_full(j):
                # One packed DMA + two full-width contiguous VectorE ops.
                alloc_mt(j)
                st = stgpool.tile([128, 5 * 4096], u8, tag="stg", name=f"st{j}")
                nc.sync.dma_start(out=st[:], in_=wbi_d[:, j - 1])
                wv = st[:, 0:8192].bitcast(bf16)
                bv = st[:, 8192:16384].bitcast(bf16)
                iv = st[:, 16384:20480]
                nc.vector.tensor_mul(out=mts[j][:], in0=wv, in1=iv)
                nc.vector.tensor_add(out=mts[j][:], in0=mts[j][:], in1=bv)

            def prep0_q(q):
                # Node block 0, n-quarter q: small packed DMA so mT[0]
                # becomes usable piecewise as bytes land.
                st = stgpool.tile([128, 5 * 1024], u8, tag="stg0", name=f"st0_{q}")
                nc.sync.dma_start(out=st[:], in_=wbi0_d[:, q])
                wv = st[:, 0:2048].bitcast(bf16).rearrange("p (et n) -> p et n", n=128)
                bv = st[:, 2048:4096].bitcast(bf16).rearrange("p (et n) -> p et n", n=128)
                iv = st[:, 4096:5120].rearrange("p (et n) -> p et n", n=128)
                dst = mts[0].rearrange("p (et n) -> p et n", n=NBLK)[
                    :, :, q * 128 : (q + 1) * 128
                ]
                nc.vector.tensor_mul(out=dst, in0=wv, in1=iv)
                nc.vector.tensor_add(out=dst, in0=dst, in1=bv)

            def group(j, bb, ft, n0, n1, tail_split=False):
                ps = mmpool.tile([128, NBLK], f32, tag="ps", name=f"ps{j}_{bb}_{ft}_{n0}")
                for et in range(ET):
                    c0 = et * F + ft * 128
                    nc.tensor.matmul(
                        ps[:, n0:n1],
                        lhsT=x_slice(bb, c0),
                        rhs=mts[j][:, et * NBLK + n0 : et * NBLK + n1],
                        start=(et == 0),
                        stop=(et == ET - 1),
                    )
                ot = opool.tile([128, NBLK], bf16, tag="o", name=f"o{j}_{bb}_{ft}_{n0}")
                orow = o_d[bb, ft * 128 : (ft + 1) * 128]
                if tail_split:
                    # Pipeline the final drain: half-drain, half-DMA-out, so
                    # the kernel tail is half a drain shorter.
                    mid = (n0 + n1) // 2
                    nc.scalar.copy(out=ot[:, n0:mid], in_=ps[:, n0:mid])
                    nc.sync.dma_start(
                        out=orow[:, j * NBLK + n0 : j * NBLK + mid], in_=ot[:, n0:mid]
                    )
                    nc.scalar.copy(out=ot[:, mid:n1], in_=ps[:, mid:n1])
                    nc.sync.dma_start(
                        out=orow[:, j * NBLK + mid : j * NBLK + n1], in_=ot[:, mid:n1]
                    )
                else:
                    nc.scalar.copy(out=ot[:, n0:n1], in_=ps[:, n0:n1])
                    nc.sync.dma_start(
                        out=orow[:, j * NBLK + n0 : j * NBLK + n1], in_=ot[:, n0:n1]
                    )

            def mms(j, b_lo=0, b_hi=B):
                for bb in range(b_lo, b_hi):
                    for ft in range(FT):
                        last = j == NJ - 1 and bb == B - 1 and ft == FT - 1
                        group(j, bb, ft, 0, NBLK, tail_split=last)

            # ---- software pipeline ----
            # PE pre-warm first (no input deps): dummy matmuls on a zeroed
            # tile keep the PE busy through the HAM activity window while the
            # first real supply streams in, so the real matmuls start at
            # K=8/8 (2.4 GHz) instead of paying the 1.2 GHz cold ramp.
            wz = stgpool.tile([128, NBLK], bf16, tag="warm", name="wz")
            nc.vector.memset(wz[:], 0.0)
            ps_warm = warmpool.tile([128, NBLK], f32, tag="psw", name="ps_warm")
            for _ in range(34):
                nc.tensor.matmul(
                    ps_warm[:], lhsT=wz[:, 0:128], rhs=wz[:], start=True, stop=True
                )
            load_x(0)
            alloc_mt(0)
            prep0_q(0)
            prep0_q(1)
            load_x(1)
            prep0_q(2)
            prep0_q(3)
            load_x(2)
            prep_full(1)
            for q in range(3, B // XG):
                load_x(q)
            # Node block 0: first batches run half-width so matmuls start as
            # soon as x group 0 + quarters 0-1 have landed; the n-halves are
            # swept separately so the second half only needs quarters 2-3.
            for bb in range(4):
                for ft in range(FT):
                    group(0, bb, ft, 0, 256)
            for bb in range(4):
                for ft in range(FT):
                    group(0, bb, ft, 256, NBLK)
            for bb in range(4, B):
                for ft in range(FT):
                    group(0, bb, ft, 0, NBLK)
            prep_full(2)
            mms(1)
            prep_full(3)
            for j in range(2, NJ):
                mms(j)
                if j + 2 < NJ:
                    prep_full(j + 2)
    nc.finalize()
    return nc


def _get_nc():
    if "nc" not in _CACHE:
        _CACHE["nc"] = _build_nc()
    return _CACHE["nc"]


def run(inputs, trace=False, tmpdir=None, trace_cores=None):
    """Shard + host-pack inputs, run the SPMD bass kernel on 8 cores,
    return (full_output, BassKernelResults)."""
    from concourse.bass_utils import run_bass_kernel_spmd

    bf16 = ml_dtypes.bfloat16
    x = np.asarray(inputs["x"], dtype=np.float32)
    w = np.asarray(inputs["w"], dtype=np.float32)
    inci = np.asarray(inputs["inci"], dtype=np.float32)
    b = np.asarray(inputs["b"], dtype=np.float32)
    assert x.shape == (B, E, F) and w.shape == (N, E)

    in_maps = []
    for c in range(NCORES):
        sl = slice(c * ESH, (c + 1) * ESH)
        wT = w[:, sl].T  # [ESH, N]
        bT = b[:, sl].T
        iT = inci[:, sl].T

        def block(a, j0, j1, nsub, dt):
            # [ESH, ncols] -> [128, nblocks, et-major bytes]
            t = a[:, j0:j1]
            nb = (j1 - j0) // nsub
            t = t.reshape(ET, 128, nb, nsub).transpose(1, 2, 0, 3)  # [128, nb, et, nsub]
            t = np.ascontiguousarray(t.astype(dt))
            return t.reshape(128, nb, -1).view(np.uint8)

        # node blocks 1..7: w|b|i packed per block, (et, n512) order
        wbi = np.concatenate(
            [
                block(wT, NBLK, N, NBLK, bf16),
                block(bT, NBLK, N, NBLK, bf16),
                block(iT, NBLK, N, NBLK, np.uint8),
            ],
            axis=2,
        )
        # node block 0 in quarters, (et, n128) order
        wbi0 = np.concatenate(
            [
                block(wT, 0, NBLK, 128, bf16),
                block(bT, 0, NBLK, 128, bf16),
                block(iT, 0, NBLK, 128, np.uint8),
            ],
            axis=2,
        )
        # x[p, b, et*F+f] = x[b, et*128+p, f]
        xp = np.ascontiguousarray(
            x[:, sl, :].reshape(B, ET, 128, F).transpose(2, 0, 1, 3).astype(bf16)
        ).reshape(128, B, ET * F)
        in_maps.append({"x": xp, "wbi": wbi, "wbi0": wbi0})

    nc = _get_nc()
    res = run_bass_kernel_spmd(
        nc,
        in_maps,
        core_ids=list(range(NCORES)),
        trace=trace,
        tmpdir=tmpdir,
        trace_cores=trace_cores,
    )
    # Sum the 8 bf16 partial products in fp32 and transpose [B,F,N]->[B,N,F].
    total = res.results[0]["out"].astype(np.float32)
    for c in range(1, NCORES):
        total = total + res.results[c]["out"].astype(np.float32)
    out = np.ascontiguousarray(total.transpose(0, 2, 1))
    return out, res


def kernel(x, inci, w, b):
    out, _ = run({"x": x, "inci": inci, "w": w, "b": b})
    return out
